# revision 1
# baseline (speedup 1.0000x reference)
"""Trainium2 Bass kernel for nn_Decoder (2-layer RNN decoder).

Reference computation (per layer, scanned over T):
    c = concat([x_t, h], 1); h' = tanh(c @ Wh + bh); o = tanh(c @ Wo + bo)
Layer 0 h0 = encoder_output, layer 1 h0 = 0, output = layer-1 o.

Strategy (per core, batch shard of 8):
  - x contributions + biases precomputed as big fp32r GEMMs, stored as a
    bf16 hi/lo pair (P = X@Whx + bh ~ P_hi + P_lo to ~fp32 precision)
  - recurrence critical path only computes h @ Whh (bf16, weight-load bound);
    P_hi/P_lo are added via bf16 identity matmuls in the same PSUM group;
    tanh on ScalarE writes the transposed hidden-state store directly
  - o outputs batched: tanh(X@Wox + H_prev@Woh + bo); these GEMMs are
    interleaved one matmul per recurrence step so they execute inside the
    tanh wait windows and keep the PE array warm (HAM at full clock)
  - everything stays in [feature, t*8+b] transposed layout; the final GEMM
    uses activations as the stationary operand to emit row-major output.

Sharding: data-parallel over batch (B=64 -> 8 cores x 8), weights replicated.
"""
import sys

if "/opt/trn_rl_repo" not in sys.path:
    sys.path.insert(0, "/opt/trn_rl_repo")

import numpy as np
from contextlib import ExitStack

import concourse.bacc as bacc
import concourse.mybir as mybir
import concourse.tile as tile
from concourse.bass_utils import run_bass_kernel_spmd
from concourse.masks import make_identity
from concourse.tile_rust import add_dep_helper

F32 = mybir.dt.float32
F32R = mybir.dt.float32r
BF16 = mybir.dt.bfloat16
Tanh = mybir.ActivationFunctionType.Tanh
Ident = mybir.ActivationFunctionType.Identity
ADD = mybir.AluOpType.add
SUB = mybir.AluOpType.subtract

B_LOC = 8          # batch per core
D = 512            # input feature dim
H = 512            # hidden dim
KC = 4             # 128-chunks in D or H
N_CORES = 8


def build_kernel(T=256):
    """Build the per-core Bass program (fully unrolled, Tile-scheduled)."""
    TB = T * B_LOC                 # time-major column count (t*8+b)
    NB = TB // 512                 # number of 512-wide TB blocks in GEMMs
    MT = TB // 128                 # number of 128-row output chunks
    HS = TB + B_LOC                # hidden store column count (h_{-1}..h_{T-1})

    nc = bacc.Bacc(None)
    x_d = nc.dram_tensor("x", [B_LOC, T, D], F32, kind="ExternalInput")
    enc_d = nc.dram_tensor("encoder_output", [B_LOC, H], F32, kind="ExternalInput")
    wh0_d = nc.dram_tensor("Wh0", [D + H, H], F32, kind="ExternalInput")
    bh0_d = nc.dram_tensor("bh0", [H], F32, kind="ExternalInput")
    wo0_d = nc.dram_tensor("Wo0", [D + H, D], F32, kind="ExternalInput")
    bo0_d = nc.dram_tensor("bo0", [D], F32, kind="ExternalInput")
    wh1_d = nc.dram_tensor("Wh1", [D + H, H], F32, kind="ExternalInput")
    bh1_d = nc.dram_tensor("bh1", [H], F32, kind="ExternalInput")
    wo1_d = nc.dram_tensor("Wo1", [D + H, D], F32, kind="ExternalInput")
    bo1_d = nc.dram_tensor("bo1", [D], F32, kind="ExternalInput")
    out_d = nc.dram_tensor("out", [B_LOC, T, D], F32, kind="ExternalOutput")

    with tile.TileContext(nc) as tc, ExitStack() as ctx:
        sb = ctx.enter_context(tc.tile_pool(name="sb", bufs=1))
        stg = ctx.enter_context(tc.tile_pool(name="stg", bufs=2))
        ps_g = ctx.enter_context(tc.tile_pool(name="ps_g", bufs=2, space="PSUM"))
        ps_t = ctx.enter_context(tc.tile_pool(name="ps_t", bufs=2, space="PSUM"))
        ps_z = ctx.enter_context(tc.tile_pool(name="ps_z", bufs=2, space="PSUM"))

        # ---------- constants ----------
        ident = sb.tile([128, 128], F32, tag="ident", name="ident")
        make_identity(nc, ident[:])
        ident_b = sb.tile([128, 128], BF16, tag="ident_b", name="ident_b")
        nc.vector.tensor_copy(ident_b[:], ident[:])
        ones_f = sb.tile([1, 128], F32, tag="ones_f", name="ones_f")
        nc.vector.memset(ones_f[:], 1.0)
        ones_r = sb.tile([1, 128], F32R, tag="ones_r", name="ones_r")
        nc.vector.tensor_copy(ones_r[:], ones_f[:])

        # ---------- biases ----------
        def load_bias_cols(dram, tag):
            t_ = sb.tile([128, KC], F32, tag=tag, name=tag)
            nc.sync.dma_start(t_[:], dram[:].rearrange("(c p) -> p c", p=128))
            return t_

        bh0 = load_bias_cols(bh0_d, "bh0")
        bo0 = load_bias_cols(bo0_d, "bo0")
        bh1 = load_bias_cols(bh1_d, "bh1")
        bo1f = sb.tile([1, 512], F32, tag="bo1f", name="bo1f")
        nc.sync.dma_start(bo1f[:], bo1_d[:].rearrange("(o n) -> o n", o=1))
        bo1r = sb.tile([1, 512], F32R, tag="bo1r", name="bo1r")
        nc.vector.tensor_copy(bo1r[:], bo1f[:])

        # ---------- weights ----------
        # layout per weight half: [128, k*512 + m*128 + col] (k = K-chunk of
        # the contraction dim, m = 128-chunk of output features)
        def load_half(dram, row0, dtype, tag):
            w = sb.tile([128, KC * 512], dtype, tag=tag, name=tag)
            s = stg.tile([128, KC * 512], F32, tag="stag", name="stag")
            for k in range(KC):
                nc.sync.dma_start(
                    s[:, k * 512:(k + 1) * 512],
                    dram[row0 + k * 128: row0 + (k + 1) * 128, :])
            nc.vector.tensor_copy(w[:], s[:])
            return w

        wx0 = load_half(wh0_d, 0, F32R, "wxA")      # Whx0 (x part, fp32r)
        whh0 = load_half(wh0_d, D, BF16, "whh0")    # Whh0 (recurrent, bf16)

        # ---------- x load + transpose to xT[k] = [128, TB] fp32r ----------
        xT = [sb.tile([128, TB], F32R, tag=f"xT{k}", name=f"xT{k}")
              for k in range(KC)]

        def load_x_block(j):
            xs = stg.tile([128, 512], F32, tag="xs", name="xs")
            nc.sync.dma_start(
                xs[:],
                x_d[:, j * 16:(j + 1) * 16, :].rearrange("b t d -> t b d"))
            for k in range(KC):
                pt = ps_t.tile([128, 128], F32, tag="pt", name="pt")
                nc.tensor.transpose(pt[:], xs[:, k * 128:(k + 1) * 128], ident[:])
                nc.vector.tensor_copy(xT[k][:, j * 128:(j + 1) * 128], pt[:])

        for j in range(min(4, MT)):
            load_x_block(j)

        # ---------- hidden-state stores [128, k*HS + col], col t = h_{t-1} ----------
        h0T = sb.tile([128, KC * HS], BF16, tag="h0T", name="h0T")
        encs = stg.tile([B_LOC, H], F32, tag="encs", name="encs")
        nc.sync.dma_start(encs[:], enc_d[:])
        for k in range(KC):
            pt = ps_t.tile([128, B_LOC], F32, tag="pt", name="pt")
            nc.tensor.transpose(pt[:], encs[:, k * 128:(k + 1) * 128],
                                ident[0:B_LOC, 0:B_LOC])
            nc.vector.tensor_copy(h0T[:, k * HS: k * HS + B_LOC], pt[:])

        def p_view(P):
            return P[:].rearrange("p (t m b) -> p t m b", m=KC, b=B_LOC)

        # ---------- P GEMM: P_hi/P_lo bf16 pair = X @ Whx + bh ----------
        def emit_p_block(Ph, Pl, w, src, bias, m, n):
            """One (m, n) block of a P GEMM; returns list of emit thunks."""
            thunks = []
            pg_box = []

            def mk_mm(k):
                def f(dep=None):
                    if k == 0:
                        pg_box.append(ps_g.tile([128, 512], F32, tag="pg",
                                                name="pg"))
                    mm = nc.tensor.matmul(
                        pg_box[0][:],
                        w[:, k * 512 + m * 128: k * 512 + (m + 1) * 128],
                        src[k][:, n * 512:(n + 1) * 512],
                        start=(k == 0), stop=(k == KC - 1))
                    if dep is not None:
                        add_dep_helper(mm.ins, dep, sync=False, reason="spread")
                return f

            for k in range(KC):
                thunks.append(mk_mm(k))

            def epi(dep=None):
                pg = pg_box[0]
                nc.scalar.activation(
                    p_view(Ph)[:, n * 64:(n + 1) * 64, m, :],
                    pg[:].rearrange("p (t b) -> p t b", b=B_LOC),
                    Ident, bias=bias[:, m: m + 1])
                nc.vector.scalar_tensor_tensor(
                    p_view(Pl)[:, n * 64:(n + 1) * 64, m, :],
                    pg[:].rearrange("p (t b) -> p t b", b=B_LOC),
                    bias[:, m: m + 1],
                    p_view(Ph)[:, n * 64:(n + 1) * 64, m, :],
                    ADD, SUB)
            thunks.append(epi)
            return thunks

        # ---------- o GEMM: tanh(X@Wox + Hprev@Woh + bo) -> fp32r [feat, TB] ----------
        def emit_o_block(dst, wx, wh, hT, bias, m, n):
            thunks = []
            pg_box = []

            def mk_x(k):
                def f(dep=None):
                    if k == 0:
                        pg_box.append(ps_g.tile([128, 512], F32, tag="pg",
                                                name="pg"))
                    mm = nc.tensor.matmul(
                        pg_box[0][:],
                        wx[:, k * 512 + m * 128: k * 512 + (m + 1) * 128],
                        xT[k][:, n * 512:(n + 1) * 512],
                        start=(k == 0), stop=False, skip_group_check=True)
                    if dep is not None:
                        add_dep_helper(mm.ins, dep, sync=False, reason="spread")
                return f

            def mk_h(k):
                def f(dep=None):
                    mm = nc.tensor.matmul(
                        pg_box[0][:],
                        wh[:, k * 512 + m * 128: k * 512 + (m + 1) * 128],
                        hT[:, k * HS + n * 512: k * HS + (n + 1) * 512],
                        start=False, stop=(k == KC - 1), skip_group_check=True)
                    if dep is not None:
                        add_dep_helper(mm.ins, dep, sync=False, reason="spread")
                return f

            for k in range(KC):
                thunks.append(mk_x(k))
            for k in range(KC):
                thunks.append(mk_h(k))

            def epi(dep=None):
                nc.scalar.activation(dst[m][:, n * 512:(n + 1) * 512],
                                     pg_box[0][:], Tanh, bias=bias[:, m: m + 1])
            thunks.append(epi)
            return thunks

        # ---------- final output block ([TB, feat] row-major) ----------
        def emit_out_block(wo1x, h1T, mt):
            thunks = []
            po_box = []

            def bias_mm(dep=None):
                po_box.append(ps_g.tile([128, 512], F32, tag="pg", name="pg"))
                mm = nc.tensor.matmul(po_box[0][:], ones_r[:], bo1r[:],
                                 start=True, stop=False, skip_group_check=True)
                if dep is not None:
                    add_dep_helper(mm.ins, dep, sync=False, reason="spread")
            thunks.append(bias_mm)

            def mk_x(k):
                def f(dep=None):
                    mm = nc.tensor.matmul(
                        po_box[0][:], out0T[k][:, mt * 128:(mt + 1) * 128],
                        wo1x[:, k * 512:(k + 1) * 512],
                        start=False, stop=False, skip_group_check=True)
                    if dep is not None:
                        add_dep_helper(mm.ins, dep, sync=False, reason="spread")
                return f

            def mk_h(k):
                def f(dep=None):
                    mm = nc.tensor.matmul(
                        po_box[0][:],
                        h1T[:, k * HS + mt * 128: k * HS + (mt + 1) * 128],
                        woh1[:, k * 512:(k + 1) * 512],
                        start=False, stop=(k == KC - 1), skip_group_check=True)
                    if dep is not None:
                        add_dep_helper(mm.ins, dep, sync=False, reason="spread")
                return f

            for k in range(KC):
                thunks.append(mk_x(k))
            for k in range(KC):
                thunks.append(mk_h(k))

            def epi(dep=None):
                orow = stg.tile([128, 512], F32, tag="orow", name="orow")
                nc.scalar.activation(orow[:], po_box[0][:], Tanh)
                nc.sync.dma_start(
                    out_d[:, mt * 16:(mt + 1) * 16, :].rearrange("b t d -> t b d"),
                    orow[:])
            thunks.append(epi)
            return thunks

        # ---------- recurrence with interleaved fill work ----------
        def run_recurrence(Ph, Pl, hTa, whh, fill_schedule):
            """fill_schedule: dict step -> list of thunks emitted after that
            step's matmuls (they execute on PE inside the tanh window)."""
            hview = hTa[:].rearrange("p (c s) -> p c s", c=KC)
            pv_h, pv_l = p_view(Ph), p_view(Pl)
            for t in range(T):
                z = ps_z.tile([128, 32], F32, tag="z", name="z")
                nc.tensor.matmul(
                    z[:].rearrange("p (m b) -> p m b", b=B_LOC),
                    ident_b[:], pv_h[:, t, :, :],
                    start=True, stop=False, skip_group_check=True)
                nc.tensor.matmul(
                    z[:].rearrange("p (m b) -> p m b", b=B_LOC),
                    ident_b[:], pv_l[:, t, :, :],
                    start=False, stop=False, skip_group_check=True)
                for k in range(KC):
                    for m in range(KC):
                        nc.tensor.matmul(
                            z[:, m * 8:(m + 1) * 8],
                            whh[:, k * 512 + m * 128: k * 512 + (m + 1) * 128],
                            hTa[:, k * HS + t * 8: k * HS + (t + 1) * 8],
                            start=False, stop=(k == KC - 1 and m == KC - 1),
                            skip_group_check=True)
                act = nc.scalar.activation(
                    hview[:, :, (t + 1) * 8:(t + 2) * 8],
                    z[:].rearrange("p (c b) -> p c b", b=B_LOC),
                    Tanh)
                for fn in fill_schedule.get(t, ()):
                    fn(act.ins)

        # ---------- phase plan ----------
        # P0 n=0 runs up-front (needed from step 0); n>=1 deferred into rec0
        P0h = sb.tile([128, T * 32], BF16, tag="Ph", name="P0h")
        P0l = sb.tile([128, T * 32], BF16, tag="Pl", name="P0l")
        for m in range(KC):
            for fn in emit_p_block(P0h, P0l, wx0, xT, bh0, m, 0):
                fn()
        for j in range(min(4, MT), MT):
            load_x_block(j)

        out0T = [sb.tile([128, TB], BF16, tag=f"o0T{m}", name=f"o0T{m}")
                 for m in range(KC)]
        P1h = sb.tile([128, T * 32], BF16, tag="P1h", name="P1h")
        P1l = sb.tile([128, T * 32], BF16, tag="P1l", name="P1l")
        # loads emitted after P0/x so their DMAs queue behind x and fill the
        # early-recurrence DMA-idle window instead of delaying the prologue
        wox0 = load_half(wo0_d, 0, F32R, "woxA")
        woh0 = load_half(wo0_d, D, BF16, "woh0")
        wx1 = load_half(wh1_d, 0, BF16, "wxB")
        wo1x = load_half(wo1_d, 0, BF16, "woxB")

        # h1T aliases the P0h slot (P0h is dead after rec0); its zero-init is
        # emitted just before rec1 so the WAR dep lands at the phase boundary
        h1T = sb.tile([128, KC * HS], BF16, tag="Ph", name="h1T")

        # rec0 fill: Zo0 blocks (need H0T cols < (n+1)*512, ready after step
        # (n+1)*64) then P1 blocks for the same n (need out0T block n).
        def make_sched(overflow):
            def sched(step_list, start, thunks):
                s = start
                for th in thunks:
                    if s < T - 1:
                        step_list.setdefault(s, []).append(th)
                    else:
                        overflow.append(th)
                    s += 1
                return s
            return sched

        fill0 = {}
        leftover0 = []
        sched0 = make_sched(leftover0)
        # deferred P0 blocks (cols n*64.. are needed only from rec0 step n*64)
        cur = 1
        for n in range(1, NB):
            for m in range(KC):
                cur = sched0(fill0, cur, emit_p_block(P0h, P0l, wx0, xT,
                                                      bh0, m, n))
        cur = max(cur, 66)
        for n in range(NB):
            cur = max(cur, (n + 1) * 64 + 1)
            for m in range(KC):
                cur = sched0(fill0, cur, emit_o_block(out0T, wox0, woh0, h0T,
                                                      bo0, m, n))
            for m in range(KC):
                cur = sched0(fill0, cur, emit_p_block(P1h, P1l, wx1, out0T,
                                                      bh1, m, n))

        run_recurrence(P0h, P0l, h0T, whh0, fill0)

        # tail Zo0/P1 blocks (depend on the last hidden states / out0 cols)
        # must complete before rec1 steps that read P1 -> run them serially
        for th in leftover0:
            th()

        whh1 = load_half(wh1_d, D, BF16, "whh1")
        woh1 = load_half(wo1_d, D, BF16, "woh1")

        for k in range(KC):
            nc.vector.memset(h1T[:, k * HS: k * HS + B_LOC], 0.0)

        # final-output blocks go into rec1 fill slots
        fill1 = {}
        leftover1 = []
        sched1 = make_sched(leftover1)
        cur = 0
        for mt in range(MT):
            cur = max(cur, (mt + 1) * 16 + 1)
            cur = sched1(fill1, cur, emit_out_block(wo1x, h1T, mt))

        run_recurrence(P1h, P1l, h1T, whh1, fill1)
        for th in leftover1:
            th()

    nc.compile()
    return nc


_NC_CACHE = {}


def _get_nc(T=256):
    if T not in _NC_CACHE:
        _NC_CACHE[T] = build_kernel(T)
    return _NC_CACHE[T]


def kernel(**inputs):
    x = np.ascontiguousarray(inputs["x"], dtype=np.float32)
    enc = np.ascontiguousarray(inputs["encoder_output"], dtype=np.float32)
    B, T, _ = x.shape
    nc = _get_nc(T)
    shared = {
        "Wh0": np.ascontiguousarray(inputs["Wh0"], np.float32),
        "bh0": np.ascontiguousarray(inputs["bh0"], np.float32),
        "Wo0": np.ascontiguousarray(inputs["Wo0"], np.float32),
        "bo0": np.ascontiguousarray(inputs["bo0"], np.float32),
        "Wh1": np.ascontiguousarray(inputs["Wh1"], np.float32),
        "bh1": np.ascontiguousarray(inputs["bh1"], np.float32),
        "Wo1": np.ascontiguousarray(inputs["Wo1"], np.float32),
        "bo1": np.ascontiguousarray(inputs["bo1"], np.float32),
    }
    in_maps = []
    for c in range(N_CORES):
        in_maps.append({
            "x": x[c * B_LOC:(c + 1) * B_LOC],
            "encoder_output": enc[c * B_LOC:(c + 1) * B_LOC],
            **shared,
        })
    res = run_bass_kernel_spmd(nc, in_maps, core_ids=list(range(N_CORES)))
    out = np.concatenate([res.results[c]["out"] for c in range(N_CORES)], axis=0)
    return out.astype(np.float32)



# revision 7
# speedup vs baseline: 3.1915x; 3.1915x over previous
"""Trainium2 Bass kernel for nn_Decoder (2-layer RNN decoder).

Reference computation (per layer, scanned over T):
    c = concat([x_t, h], 1); h' = tanh(c @ Wh + bh); o = tanh(c @ Wo + bo)
Layer 0 h0 = encoder_output, layer 1 h0 = 0, output = layer-1 o.

Strategy (per core, batch shard of 8):
  - x contributions + biases precomputed as big fp32r GEMMs, stored as a
    bf16 hi/lo pair (P = X@Whx + bh ~ P_hi + P_lo to ~fp32 precision)
  - recurrence critical path only computes h @ Whh (bf16, weight-load bound);
    P_hi/P_lo are added via bf16 identity matmuls in the same PSUM group;
    tanh on ScalarE writes the transposed hidden-state store directly
  - o outputs batched: tanh(X@Wox + H_prev@Woh + bo); these GEMMs are
    interleaved one matmul per recurrence step so they execute inside the
    tanh wait windows and keep the PE array warm (HAM at full clock)
  - everything stays in [feature, t*8+b] transposed layout; the final GEMM
    uses activations as the stationary operand to emit row-major output.

Sharding: data-parallel over batch (B=64 -> 8 cores x 8), weights replicated.
"""
import sys

if "/opt/trn_rl_repo" not in sys.path:
    sys.path.insert(0, "/opt/trn_rl_repo")

import numpy as np
from contextlib import ExitStack

import concourse.bacc as bacc
import concourse.mybir as mybir
import concourse.tile as tile
from concourse.bass_utils import run_bass_kernel_spmd
from concourse.masks import make_identity
from concourse.tile_rust import add_dep_helper

F32 = mybir.dt.float32
F32R = mybir.dt.float32r
BF16 = mybir.dt.bfloat16
Tanh = mybir.ActivationFunctionType.Tanh
Ident = mybir.ActivationFunctionType.Identity
ADD = mybir.AluOpType.add
SUB = mybir.AluOpType.subtract

B_LOC = 8          # batch per core
D = 512            # input feature dim
H = 512            # hidden dim
KC = 4             # 128-chunks in D or H
N_CORES = 8


def build_kernel(T=256):
    """Build the per-core Bass program (fully unrolled, Tile-scheduled)."""
    TB = T * B_LOC                 # time-major column count (t*8+b)
    NB = TB // 512                 # number of 512-wide TB blocks in GEMMs
    MT = TB // 128                 # number of 128-row output chunks
    HS = TB + B_LOC                # hidden store column count (h_{-1}..h_{T-1})

    nc = bacc.Bacc(None)
    x_d = nc.dram_tensor("x", [B_LOC, T, D], F32, kind="ExternalInput")
    enc_d = nc.dram_tensor("encoder_output", [B_LOC, H], F32, kind="ExternalInput")
    wh0_d = nc.dram_tensor("Wh0", [D + H, H], F32, kind="ExternalInput")
    bh0_d = nc.dram_tensor("bh0", [H], F32, kind="ExternalInput")
    wo0_d = nc.dram_tensor("Wo0", [D + H, D], F32, kind="ExternalInput")
    bo0_d = nc.dram_tensor("bo0", [D], F32, kind="ExternalInput")
    wh1_d = nc.dram_tensor("Wh1", [D + H, H], F32, kind="ExternalInput")
    bh1_d = nc.dram_tensor("bh1", [H], F32, kind="ExternalInput")
    wo1_d = nc.dram_tensor("Wo1", [D + H, D], F32, kind="ExternalInput")
    bo1_d = nc.dram_tensor("bo1", [D], F32, kind="ExternalInput")
    out_d = nc.dram_tensor("out", [B_LOC, T, D], F32, kind="ExternalOutput")

    with tile.TileContext(nc) as tc, ExitStack() as ctx:
        sb = ctx.enter_context(tc.tile_pool(name="sb", bufs=1))
        stg = ctx.enter_context(tc.tile_pool(name="stg", bufs=2))
        ps_g = ctx.enter_context(tc.tile_pool(name="ps_g", bufs=2, space="PSUM"))
        ps_t = ctx.enter_context(tc.tile_pool(name="ps_t", bufs=2, space="PSUM"))
        ps_z = ctx.enter_context(tc.tile_pool(name="ps_z", bufs=2, space="PSUM"))

        # ---------- constants ----------
        ident = sb.tile([128, 128], F32, tag="ident", name="ident")
        make_identity(nc, ident[:])
        ident_b = sb.tile([128, 128], BF16, tag="ident_b", name="ident_b")
        nc.vector.tensor_copy(ident_b[:], ident[:])
        ones_f = sb.tile([1, 128], F32, tag="ones_f", name="ones_f")
        nc.vector.memset(ones_f[:], 1.0)
        ones_r = sb.tile([1, 128], BF16, tag="ones_r", name="ones_r")
        nc.vector.tensor_copy(ones_r[:], ones_f[:])

        # ---------- biases ----------
        def load_bias_cols(dram, tag):
            t_ = sb.tile([128, KC], F32, tag=tag, name=tag)
            nc.sync.dma_start(t_[:], dram[:].rearrange("(c p) -> p c", p=128))
            return t_

        bh0 = load_bias_cols(bh0_d, "bh0")
        bo0 = load_bias_cols(bo0_d, "bo0")
        bh1 = load_bias_cols(bh1_d, "bh1")
        bo1f = sb.tile([1, 512], F32, tag="bo1f", name="bo1f")
        nc.sync.dma_start(bo1f[:], bo1_d[:].rearrange("(o n) -> o n", o=1))
        bo1r = sb.tile([1, 512], BF16, tag="bo1r", name="bo1r")
        nc.vector.tensor_copy(bo1r[:], bo1f[:])

        # ---------- weights ----------
        # layout per weight half: [128, k*512 + m*128 + col] (k = K-chunk of
        # the contraction dim, m = 128-chunk of output features)
        def load_half(dram, row0, dtype, tag):
            w = sb.tile([128, KC * 512], dtype, tag=tag, name=tag)
            s = stg.tile([128, KC * 512], F32, tag="stag", name="stag")
            for k in range(KC):
                nc.sync.dma_start(
                    s[:, k * 512:(k + 1) * 512],
                    dram[row0 + k * 128: row0 + (k + 1) * 128, :])
            nc.vector.tensor_copy(w[:], s[:])
            return w

        wx0 = load_half(wh0_d, 0, BF16, "wxA")      # Whx0 (x part, bf16)
        whh0 = load_half(wh0_d, D, BF16, "whh0")    # Whh0 (recurrent, bf16)

        # ---------- x load + transpose to xT[k] = [128, TB] bf16 ----------
        xT = [sb.tile([128, TB], BF16, tag=f"xT{k}", name=f"xT{k}")
              for k in range(KC)]

        def load_x_block(j):
            xs = stg.tile([128, 512], F32, tag="xs", name="xs")
            nc.sync.dma_start(
                xs[:],
                x_d[:, j * 16:(j + 1) * 16, :].rearrange("b t d -> t b d"))
            xsb = stg.tile([128, 512], BF16, tag="xsb", name="xsb")
            nc.vector.tensor_copy(xsb[:], xs[:])
            for k in range(KC):
                pt = ps_t.tile([128, 128], BF16, tag="pt", name="pt")
                nc.tensor.transpose(pt[:], xsb[:, k * 128:(k + 1) * 128],
                                    ident_b[:])
                nc.vector.tensor_copy(xT[k][:, j * 128:(j + 1) * 128], pt[:])

        for j in range(min(4, MT)):
            load_x_block(j)

        # ---------- hidden-state stores [128, k*HS + col], col t = h_{t-1} ----------
        h0T = sb.tile([128, KC * HS], BF16, tag="h0T", name="h0T")
        encs = stg.tile([B_LOC, H], F32, tag="encs", name="encs")
        nc.sync.dma_start(encs[:], enc_d[:])
        encsb = stg.tile([B_LOC, H], BF16, tag="encsb", name="encsb")
        nc.vector.tensor_copy(encsb[:], encs[:])
        for k in range(KC):
            pt = ps_t.tile([128, B_LOC], BF16, tag="pt", name="pt")
            nc.tensor.transpose(pt[:], encsb[:, k * 128:(k + 1) * 128],
                                ident_b[0:B_LOC, 0:B_LOC])
            nc.vector.tensor_copy(h0T[:, k * HS: k * HS + B_LOC], pt[:])

        def p_view(P):
            return P[:].rearrange("p (t m b) -> p t m b", m=KC, b=B_LOC)

        # ---------- P GEMM: P_hi/P_lo bf16 pair = X @ Whx + bh ----------
        def emit_p_block(Ph, Pl, w, src, bias, m, n):
            """One (m, n) block of a P GEMM; returns list of emit thunks."""
            thunks = []
            pg_box = []

            def mk_mm(k):
                def f(dep=None):
                    if k == 0:
                        pg_box.append(ps_g.tile([128, 512], F32, tag="pg",
                                                name="pg"))
                    mm = nc.tensor.matmul(
                        pg_box[0][:],
                        w[:, k * 512 + m * 128: k * 512 + (m + 1) * 128],
                        src[k][:, n * 512:(n + 1) * 512],
                        start=(k == 0), stop=(k == KC - 1))
                    if dep is not None:
                        add_dep_helper(mm.ins, dep, sync=False, reason="spread")
                return f

            for k in range(KC):
                thunks.append(mk_mm(k))

            def epi(dep=None):
                pg = pg_box[0]
                nc.scalar.activation(
                    p_view(Ph)[:, n * 64:(n + 1) * 64, m, :],
                    pg[:].rearrange("p (t b) -> p t b", b=B_LOC),
                    Ident, bias=bias[:, m: m + 1])
                nc.vector.scalar_tensor_tensor(
                    p_view(Pl)[:, n * 64:(n + 1) * 64, m, :],
                    pg[:].rearrange("p (t b) -> p t b", b=B_LOC),
                    bias[:, m: m + 1],
                    p_view(Ph)[:, n * 64:(n + 1) * 64, m, :],
                    ADD, SUB)
            thunks.append(epi)
            return thunks

        # ---------- o GEMM: tanh(X@Wox + Hprev@Woh + bo) -> fp32r [feat, TB] ----------
        def emit_o_block(dst, wx, wh, hT, bias, m, n):
            thunks = []
            pg_box = []

            def mk_x(k):
                def f(dep=None):
                    if k == 0:
                        pg_box.append(ps_g.tile([128, 512], F32, tag="pg",
                                                name="pg"))
                    mm = nc.tensor.matmul(
                        pg_box[0][:],
                        wx[:, k * 512 + m * 128: k * 512 + (m + 1) * 128],
                        xT[k][:, n * 512:(n + 1) * 512],
                        start=(k == 0), stop=False, skip_group_check=True)
                    if dep is not None:
                        add_dep_helper(mm.ins, dep, sync=False, reason="spread")
                return f

            def mk_h(k):
                def f(dep=None):
                    mm = nc.tensor.matmul(
                        pg_box[0][:],
                        wh[:, k * 512 + m * 128: k * 512 + (m + 1) * 128],
                        hT[:, k * HS + n * 512: k * HS + (n + 1) * 512],
                        start=False, stop=(k == KC - 1), skip_group_check=True)
                    if dep is not None:
                        add_dep_helper(mm.ins, dep, sync=False, reason="spread")
                return f

            for k in range(KC):
                thunks.append(mk_x(k))
            for k in range(KC):
                thunks.append(mk_h(k))

            def epi(dep=None):
                nc.scalar.activation(dst[m][:, n * 512:(n + 1) * 512],
                                     pg_box[0][:], Tanh, bias=bias[:, m: m + 1])
            thunks.append(epi)
            return thunks

        # ---------- final output block ([TB, feat] row-major) ----------
        def emit_out_block(wo1x, h1T, mt):
            thunks = []
            po_box = []

            def bias_mm(dep=None):
                po_box.append(ps_g.tile([128, 512], F32, tag="pg", name="pg"))
                mm = nc.tensor.matmul(po_box[0][:], ones_r[:], bo1r[:],
                                 start=True, stop=False, skip_group_check=True)
                if dep is not None:
                    add_dep_helper(mm.ins, dep, sync=False, reason="spread")
            thunks.append(bias_mm)

            def mk_x(k):
                def f(dep=None):
                    mm = nc.tensor.matmul(
                        po_box[0][:], out0T[k][:, mt * 128:(mt + 1) * 128],
                        wo1x[:, k * 512:(k + 1) * 512],
                        start=False, stop=False, skip_group_check=True)
                    if dep is not None:
                        add_dep_helper(mm.ins, dep, sync=False, reason="spread")
                return f

            def mk_h(k):
                def f(dep=None):
                    mm = nc.tensor.matmul(
                        po_box[0][:],
                        h1T[:, k * HS + mt * 128: k * HS + (mt + 1) * 128],
                        woh1[:, k * 512:(k + 1) * 512],
                        start=False, stop=(k == KC - 1), skip_group_check=True)
                    if dep is not None:
                        add_dep_helper(mm.ins, dep, sync=False, reason="spread")
                return f

            for k in range(KC):
                thunks.append(mk_x(k))
            for k in range(KC):
                thunks.append(mk_h(k))

            def epi(dep=None):
                orow = stg.tile([128, 512], F32, tag="orow", name="orow")
                nc.scalar.activation(orow[:], po_box[0][:], Tanh)
                nc.sync.dma_start(
                    out_d[:, mt * 16:(mt + 1) * 16, :].rearrange("b t d -> t b d"),
                    orow[:])
            thunks.append(epi)
            return thunks

        # ---------- recurrence with interleaved fill work ----------
        def run_recurrence(Ph, Pl, hTa, whh, fill_schedule):
            """fill_schedule: dict step -> list of thunks emitted after that
            step's matmuls (they execute on PE inside the tanh window)."""
            hview = hTa[:].rearrange("p (c s) -> p c s", c=KC)
            pv_h, pv_l = p_view(Ph), p_view(Pl)
            for t in range(T):
                z = ps_z.tile([128, 32], F32, tag="z", name="z")
                nc.tensor.matmul(
                    z[:].rearrange("p (m b) -> p m b", b=B_LOC),
                    ident_b[:], pv_h[:, t, :, :],
                    start=True, stop=False, skip_group_check=True)
                nc.tensor.matmul(
                    z[:].rearrange("p (m b) -> p m b", b=B_LOC),
                    ident_b[:], pv_l[:, t, :, :],
                    start=False, stop=False, skip_group_check=True)
                for k in range(KC):
                    for m in range(KC):
                        nc.tensor.matmul(
                            z[:, m * 8:(m + 1) * 8],
                            whh[:, k * 512 + m * 128: k * 512 + (m + 1) * 128],
                            hTa[:, k * HS + t * 8: k * HS + (t + 1) * 8],
                            start=False, stop=(k == KC - 1 and m == KC - 1),
                            skip_group_check=True)
                act = nc.scalar.activation(
                    hview[:, :, (t + 1) * 8:(t + 2) * 8],
                    z[:].rearrange("p (c b) -> p c b", b=B_LOC),
                    Tanh)
                for fn in fill_schedule.get(t, ()):
                    fn(act.ins)

        # ---------- phase plan ----------
        # P0 n=0 runs up-front (needed from step 0); n>=1 deferred into rec0
        P0h = sb.tile([128, T * 32], BF16, tag="Ph", name="P0h")
        P0l = sb.tile([128, T * 32], BF16, tag="Pl", name="P0l")
        for m in range(KC):
            for fn in emit_p_block(P0h, P0l, wx0, xT, bh0, m, 0):
                fn()
        for j in range(min(4, MT), MT):
            load_x_block(j)

        out0T = [sb.tile([128, TB], BF16, tag=f"o0T{m}", name=f"o0T{m}")
                 for m in range(KC)]
        P1h = sb.tile([128, T * 32], BF16, tag="P1h", name="P1h")
        P1l = sb.tile([128, T * 32], BF16, tag="P1l", name="P1l")
        # loads emitted after P0/x so their DMAs queue behind x and fill the
        # early-recurrence DMA-idle window instead of delaying the prologue
        wox0 = load_half(wo0_d, 0, BF16, "woxA")
        woh0 = load_half(wo0_d, D, BF16, "woh0")
        wx1 = load_half(wh1_d, 0, BF16, "wxB")
        wo1x = load_half(wo1_d, 0, BF16, "woxB")

        # h1T aliases the P0h slot (P0h is dead after rec0); its zero-init is
        # emitted just before rec1 so the WAR dep lands at the phase boundary
        h1T = sb.tile([128, KC * HS], BF16, tag="Ph", name="h1T")

        # rec0 fill: Zo0 blocks (need H0T cols < (n+1)*512, ready after step
        # (n+1)*64) then P1 blocks for the same n (need out0T block n).
        def make_sched(overflow):
            def sched(step_list, start, thunks):
                s = start
                for th in thunks:
                    if s < T - 1:
                        step_list.setdefault(s, []).append(th)
                    else:
                        overflow.append(th)
                    s += 1
                return s
            return sched

        fill0 = {}
        leftover0 = []
        sched0 = make_sched(leftover0)
        # deferred P0 blocks (cols n*64.. are needed only from rec0 step n*64)
        cur = 1
        for n in range(1, NB):
            for m in range(KC):
                cur = sched0(fill0, cur, emit_p_block(P0h, P0l, wx0, xT,
                                                      bh0, m, n))
        cur = max(cur, 66)
        for n in range(NB):
            cur = max(cur, (n + 1) * 64 + 1)
            for m in range(KC):
                cur = sched0(fill0, cur, emit_o_block(out0T, wox0, woh0, h0T,
                                                      bo0, m, n))
            for m in range(KC):
                cur = sched0(fill0, cur, emit_p_block(P1h, P1l, wx1, out0T,
                                                      bh1, m, n))

        run_recurrence(P0h, P0l, h0T, whh0, fill0)

        # tail Zo0/P1 blocks (depend on the last hidden states / out0 cols)
        # must complete before rec1 steps that read P1 -> run them serially
        for th in leftover0:
            th()

        whh1 = load_half(wh1_d, D, BF16, "whh1")
        woh1 = load_half(wo1_d, D, BF16, "woh1")

        for k in range(KC):
            nc.vector.memset(h1T[:, k * HS: k * HS + B_LOC], 0.0)

        # final-output blocks go into rec1 fill slots
        fill1 = {}
        leftover1 = []
        sched1 = make_sched(leftover1)
        cur = 0
        for mt in range(MT):
            cur = max(cur, (mt + 1) * 16 + 1)
            cur = sched1(fill1, cur, emit_out_block(wo1x, h1T, mt))

        run_recurrence(P1h, P1l, h1T, whh1, fill1)
        for th in leftover1:
            th()

    nc.compile()
    return nc


_NC_CACHE = {}


def _get_nc(T=256):
    if T not in _NC_CACHE:
        _NC_CACHE[T] = build_kernel(T)
    return _NC_CACHE[T]


def kernel(**inputs):
    x = np.ascontiguousarray(inputs["x"], dtype=np.float32)
    enc = np.ascontiguousarray(inputs["encoder_output"], dtype=np.float32)
    B, T, _ = x.shape
    nc = _get_nc(T)
    shared = {
        "Wh0": np.ascontiguousarray(inputs["Wh0"], np.float32),
        "bh0": np.ascontiguousarray(inputs["bh0"], np.float32),
        "Wo0": np.ascontiguousarray(inputs["Wo0"], np.float32),
        "bo0": np.ascontiguousarray(inputs["bo0"], np.float32),
        "Wh1": np.ascontiguousarray(inputs["Wh1"], np.float32),
        "bh1": np.ascontiguousarray(inputs["bh1"], np.float32),
        "Wo1": np.ascontiguousarray(inputs["Wo1"], np.float32),
        "bo1": np.ascontiguousarray(inputs["bo1"], np.float32),
    }
    in_maps = []
    for c in range(N_CORES):
        in_maps.append({
            "x": x[c * B_LOC:(c + 1) * B_LOC],
            "encoder_output": enc[c * B_LOC:(c + 1) * B_LOC],
            **shared,
        })
    res = run_bass_kernel_spmd(nc, in_maps, core_ids=list(range(N_CORES)))
    out = np.concatenate([res.results[c]["out"] for c in range(N_CORES)], axis=0)
    return out.astype(np.float32)

